# revision 14
# baseline (speedup 1.0000x reference)
"""Trainium2 Bass kernel for nn_HadamardBlock (GNN message passing block).

Reference computation (see reference.py):
    h_res = residual_layer(h, w_pre0, w_pre1)            # (nAtoms, E)
    mlp_bf = bf @ w_bf                                   # (nEdges, E)
    x = h_res[idx_s] * mlp_bf                            # gather + Hadamard
    x2 = segment_sum(x, idx_t, nAtoms) * scale_sum
    out = MLP(x2)   # Dense+ScaledSiLU then 3 residual blocks

Distribution strategy (8 cores, SPMD):
  - Edges are sharded by OWNER OF TARGET ATOM (atom ranges of 6250/core),
    so segment_sum is fully core-local and the atom MLP is data-parallel.
  - Phase 1 (h_res table) is sharded: each core computes 13 of the 104
    padded 512-atom tiles and an HBM-HBM AllGather replicates the full
    (53248, 128) bf16 table to every core.
  - Edge features ship as int8 (bf quantized by *127; 1/127 folded into
    w_bf) and are converted int8->bf16 on the vector engine on device.
  - The source gather h_res[idx_s] uses DMA gather (int16 indices; the
    table is addressed in two halves split at row 32768, and each core's
    edge stream is grouped low-half-first so indices fit in int16).
  - segment_sum runs on the tensor engine as x2^T += x^T @ onehot over
    128-atom windows; onehot is built by one DVE tensor_scalar(is_equal)
    per 128-edge block against an iota constant.
  - Per-(window, half) edge slot capacities are FIXED (1536 low / 896
    high, ~9 sigma above the binomial mean for uniform random targets),
    so the compiled program is independent of the index data and the
    NEFF cache always hits.  Falls back to data-driven capacities if an
    adversarial index distribution overflows them.

Everything is sized to minimize bytes shipped through the axon tunnel:
host->device upload is the dominant cost of a run in this environment
(~70 MB/s), not device execution (~1 ms).
"""

import math
import os
import sys
from contextlib import ExitStack

import numpy as np

for _p in ("/opt/trn_rl_repo", "/root/.axon_site/_ro/trn_rl_repo"):
    if os.path.isdir(_p) and _p not in sys.path:
        sys.path.insert(0, _p)

import ml_dtypes

import concourse.bacc as bacc
import concourse.bass as bass
import concourse.mybir as mybir
import concourse.tile as tile
from concourse.bass_utils import run_bass_kernel_spmd

BF16 = ml_dtypes.bfloat16
F32 = np.float32

P = 128
NA = 50000          # atoms
NE = 800000         # edges
EMB = 128
NCORE = 8
APC = NA // NCORE   # atoms per core = 6250
WIN = 128           # scatter window (atoms) = onehot width
NWIN = (APC + WIN - 1) // WIN           # 49 windows/core
TILE = 512
TPC = 13            # phase-1 tiles per core (104 total >= 98 real)
NAPC = TPC * TILE   # 6656 atom slots computed per core
NAPG = NCORE * NAPC  # 53248 global padded table rows
TBL_SPLIT = 32768   # table row split so int16 gather indices stay in range
LCAP_FIX = 1536     # fixed per-window slot capacity, low table half
HCAP_FIX = 896      # fixed per-window slot capacity, high table half
GCH = 64            # gather/bfT chunk size in 128-edge blocks
QBF = 127.0         # bf int8 quantization scale
QH = 40.0           # h int8 quantization scale (h^T/S clipped at +-3.175)
SILU_S = 1.0 / 0.6
INV_SQRT2 = float(1.0 / math.sqrt(2.0))

dt = mybir.dt


def _ceil128(x):
    return (np.asarray(x, np.int64) + 127) // 128 * 128


def _atom_perm(a):
    """Atom id -> physical row in the h_res DRAM table.

    Phase 1 stores each 512-atom tile via 4 PE transposes packed contiguously
    per partition; row q = tile*512 + (r%128)*4 + r//128 for r = a%512."""
    a = np.asarray(a, np.int64)
    i, r = a // 512, a % 512
    return i * 512 + (r % 128) * 4 + r // 128


def pack_edges(idx_s, idx_t):
    """Host-side edge sharding/padding. Returns static structure (identical
    across cores) + per-core slot assignment of every real edge."""
    idx_s = np.asarray(idx_s, np.int64)
    idx_t = np.asarray(idx_t, np.int64)
    core = idx_t // APC
    tloc = idx_t - core * APC
    w = tloc // WIN
    trel = tloc - w * WIN
    pi = _atom_perm(idx_s)
    g = (pi >= TBL_SPLIT).astype(np.int64)

    key = (core * 2 + g) * NWIN + w
    order = np.argsort(key, kind="stable")
    cnt = np.bincount(key, minlength=NCORE * 2 * NWIN).reshape(NCORE, 2, NWIN)

    lmax, hmax = cnt[:, 0, :].max(), cnt[:, 1, :].max()
    if lmax <= LCAP_FIX and hmax <= HCAP_FIX:
        # fixed capacities -> input-independent program structure
        LCAP = np.full(NWIN, LCAP_FIX, np.int64)
        HCAP = np.full(NWIN, HCAP_FIX, np.int64)
    else:  # pathological index distribution: fall back to data-driven caps
        LCAP = np.maximum(_ceil128(cnt[:, 0, :].max(axis=0)), 128)
        HCAP = np.maximum(_ceil128(cnt[:, 1, :].max(axis=0)), 128)

    low_off = np.concatenate([[0], np.cumsum(LCAP)])
    HBASE = int(low_off[-1])
    high_off = HBASE + np.concatenate([[0], np.cumsum(HCAP)])
    EPAD = int(high_off[-1])

    off_by_key = np.empty(NCORE * 2 * NWIN, np.int64)
    for c in range(NCORE):
        off_by_key[(c * 2 + 0) * NWIN:(c * 2 + 1) * NWIN] = low_off[:-1]
        off_by_key[(c * 2 + 1) * NWIN:(c * 2 + 2) * NWIN] = high_off[:-1]
    grp_start = np.concatenate([[0], np.cumsum(cnt.reshape(-1))])
    k_sorted = key[order]
    pos = np.arange(NE, dtype=np.int64) - grp_start[k_sorted]
    # slot in ORIGINAL edge order (avoids materializing permuted copies of
    # the big edge-feature array later)
    slot = np.empty(NE, np.int64)
    slot[order] = off_by_key[k_sorted] + pos

    return dict(
        core=core, slot=slot, pi=pi, g=g, trel=trel,
        LCAP=LCAP.astype(int), HCAP=HCAP.astype(int),
        EPAD=EPAD, HBASE=HBASE, NBLK=EPAD // 128,
    )


def build_host_inputs(h, bf, w_bf, w_pre, w_mlp1, w_res, scale_sum, pk):
    """Build the per-core in_maps (numpy arrays keyed by DRAM tensor name)."""
    S = SILU_S
    EPAD, NBLK = pk["EPAD"], pk["NBLK"]

    # folded weights, natural [in, out] layout; 10 slots of [128,128]:
    #  0: W0' = S*w_pre0       1: W1' = S*w_pre1
    #  2: Wm' = S*C*scale*w_mlp1        3: w_bf/QBF (bf int8 dequant folded)
    #  4..9: Ai' = S*w_res[i,0], Bi' = S*w_res[i,1]
    scale = float(np.asarray(scale_sum).reshape(-1)[0])
    wl = [
        np.asarray(w_pre[0], F32) * S,
        np.asarray(w_pre[1], F32) * S,
        np.asarray(w_mlp1, F32) * (S * INV_SQRT2 * scale),
        np.asarray(w_bf, F32) * (1.0 / QBF),
    ]
    for i in range(3):
        wl.append(np.asarray(w_res[i, 0], F32) * S)
        wl.append(np.asarray(w_res[i, 1], F32) * S)
    wts = np.concatenate([x.astype(BF16) for x in wl], axis=1)  # [128, 10*128]

    # h^T/S quantized to int8 at fixed scale QH (clips |h| beyond ~5.3 sigma)
    hq = np.zeros((P, NAPG), np.int8)
    hq[:, :NA] = np.clip(
        np.rint(np.asarray(h, F32).T * (QH / S)), -127, 127).astype(np.int8)

    iota = np.ascontiguousarray(
        np.broadcast_to(np.arange(WIN, dtype=F32).astype(BF16), (P, WIN)))
    ident = np.eye(P, dtype=BF16)

    # bf -> int8 in chunks (values in [0,1); round(bf*127) fits exactly);
    # chunking keeps the f32 temporary small on the cold path
    bf = np.asarray(bf, F32)
    bf_q = np.empty((NE, P), np.int8)
    tmp = np.empty((100000, P), F32)
    for s in range(0, NE, 100000):
        e = min(s + 100000, NE)
        t = tmp[:e - s]
        np.multiply(bf[s:e], QBF, out=t)
        t += 0.5
        bf_q[s:e] = t.astype(np.int8)

    ecore, slot = pk["core"], pk["slot"]
    bfr = np.zeros((NCORE, EPAD, P), np.int8)
    bfr[ecore, slot] = bf_q

    gidx = np.zeros((NCORE, EPAD), np.int16)
    gidx[ecore, slot] = (pk["pi"] - pk["g"] * TBL_SPLIT).astype(np.int16)
    gidx = np.ascontiguousarray(
        gidx.reshape(NCORE, EPAD // 16, 16).transpose(0, 2, 1))  # [NCORE,16,EPAD//16]

    tcol = np.zeros((NCORE, EPAD), BF16)
    tcol[ecore, slot] = pk["trel"].astype(BF16)
    tcol = tcol.reshape(NCORE, NBLK, P)

    # single bf16 aux tensor: wts | iota | ident | tcol  -> one device_put
    in_maps = []
    for c in range(NCORE):
        aux = np.empty((P, 10 * P + WIN + P + NBLK), BF16)
        aux[:, :10 * P] = wts
        aux[:, 10 * P:10 * P + WIN] = iota
        aux[:, 10 * P + WIN:10 * P + WIN + P] = ident
        np.copyto(aux[:, 10 * P + WIN + P:], tcol[c].T)
        bfti = np.empty((P, EPAD + NAPC), np.int8)
        np.copyto(bfti[:, :EPAD], bfr[c].T)
        bfti[:, EPAD:] = hq[:, c * NAPC:(c + 1) * NAPC]
        in_maps.append({
            "aux": aux, "bfti": bfti,
            "gidx": np.ascontiguousarray(gidx[c]),
        })
    return in_maps


def blocks_static(pk):
    """Static per-block schedule: list of (seg, w, start, stop)."""
    blocks = []
    for seg, CAPS in ((0, pk["LCAP"]), (1, pk["HCAP"])):
        for w in range(NWIN):
            nb = CAPS[w] // 128
            for j in range(nb):
                blocks.append((seg, w, j == 0, j == nb - 1))
    return blocks


def chunks_static(pk):
    """Gather/bfT chunk list: (seg, b0, b1) block ranges within one table
    half, at most GCH blocks each."""
    blocks = blocks_static(pk)
    chunks = []
    b = 0
    while b < len(blocks):
        seg = blocks[b][0]
        e = b
        while e < len(blocks) and blocks[e][0] == seg and e - b < GCH:
            e += 1
        chunks.append((seg, b, e))
        b = e
    return chunks


def build_bass(pk, enable_asserts=False, act_fn=None):
    EPAD, NBLK = pk["EPAD"], pk["NBLK"]
    blocks = blocks_static(pk)
    chunks = chunks_static(pk)
    ACT = act_fn or mybir.ActivationFunctionType.Silu

    nc = bacc.Bacc("TRN2", target_bir_lowering=False, debug=False,
                   enable_asserts=enable_asserts, num_devices=NCORE)

    AUXW = 10 * P + WIN + P + NBLK
    aux = nc.dram_tensor("aux", [P, AUXW], dt.bfloat16, kind="ExternalInput").ap()
    bfti = nc.dram_tensor("bfti", [P, EPAD + NAPC], dt.int8,
                          kind="ExternalInput").ap()
    gidx = nc.dram_tensor("gidx", [16, EPAD // 16], dt.int16,
                          kind="ExternalInput").ap()
    outt = nc.dram_tensor("outt", [P, NWIN * WIN], dt.bfloat16,
                          kind="ExternalOutput").ap()

    with tile.TileContext(nc) as tc, ExitStack() as ctx:
        const = ctx.enter_context(tc.tile_pool(name="const", bufs=1))
        dram = ctx.enter_context(tc.tile_pool(name="dram", bufs=1, space="DRAM"))
        ph1 = ctx.enter_context(tc.tile_pool(name="ph1", bufs=3))
        edge = ctx.enter_context(tc.tile_pool(name="edge", bufs=2))
        xoh = ctx.enter_context(tc.tile_pool(name="xoh", bufs=4))
        mlp = ctx.enter_context(tc.tile_pool(name="mlp", bufs=2))
        psA = ctx.enter_context(tc.tile_pool(name="psA", bufs=2, space="PSUM"))
        psT = ctx.enter_context(tc.tile_pool(name="psT", bufs=2, space="PSUM"))
        psM = ctx.enter_context(tc.tile_pool(name="psM", bufs=2, space="PSUM"))
        psX = ctx.enter_context(tc.tile_pool(name="psX", bufs=2, space="PSUM"))

        # resident constants / streams (one DMA for the whole aux block)
        aux_sb = const.tile([P, AUXW], dt.bfloat16)
        nc.sync.dma_start(aux_sb[:], aux[:])
        W = [aux_sb[:, i * P:(i + 1) * P] for i in range(10)]
        W0p, W1p, Wmp, Wbf = W[0], W[1], W[2], W[3]
        iota_sb = aux_sb[:, 10 * P:10 * P + WIN]
        ident_sb = aux_sb[:, 10 * P + WIN:10 * P + WIN + P]
        tcol16 = aux_sb[:, 10 * P + WIN + P:AUXW]
        tcol_sb = const.tile([P, NBLK], dt.float32)
        nc.vector.tensor_copy(tcol_sb[:], tcol16)
        # gather indices arrive 16-wrapped; replicate to the 128-partition
        # layout the SWDGE gather engine expects
        gidx_sb = const.tile([P, EPAD // 16], dt.int16)
        for k in range(8):
            nc.sync.dma_start(gidx_sb[16 * k:16 * (k + 1), :], gidx[:])
        staging = const.tile([P, NWIN * WIN], dt.bfloat16)

        agin = dram.tile([NAPC, P], dt.bfloat16, tag="agin")
        table = dram.tile([NAPG, P], dt.bfloat16, tag="table")

        # -------- phase 1: h_res table (sharded + AllGather) ---------------
        for i in range(TPC):
            h8 = ph1.tile([P, 512], dt.int8, tag="h8", name=f"h8_{i}")
            nc.sync.dma_start(
                h8[:], bfti[:, EPAD + i * 512:EPAD + (i + 1) * 512])
            hT = ph1.tile([P, 512], dt.bfloat16, tag="hT", name=f"hT{i}")
            nc.vector.tensor_scalar(hT[:], h8[:], 1.0 / QH, None,
                                    mybir.AluOpType.mult)
            p1 = psA.tile([P, 512], dt.float32, tag="p1", name=f"p1_{i}")
            nc.tensor.matmul(p1[:], W0p, hT[:], start=True, stop=True)
            y1 = ph1.tile([P, 512], dt.bfloat16, tag="y1", name=f"y1_{i}")
            nc.scalar.activation(y1[:], p1[:], ACT)
            p2 = psA.tile([P, 512], dt.float32, tag="p1", name=f"p2_{i}")
            nc.tensor.matmul(p2[:], W1p, y1[:], start=True, stop=True)
            y2 = ph1.tile([P, 512], dt.bfloat16, tag="y2", name=f"y2_{i}")
            nc.scalar.activation(y2[:], p2[:], ACT)
            tres = ph1.tile([P, 512], dt.bfloat16, tag="tres", name=f"tr_{i}")
            nc.vector.tensor_add(tres[:], hT[:], y2[:])
            tp = psT.tile([P, 512], dt.bfloat16, tag="tp", name=f"tp_{i}")
            for t in range(4):
                nc.tensor.transpose(tp[:, t * P:(t + 1) * P],
                                    tres[:, t * P:(t + 1) * P], ident_sb)
            st = ph1.tile([P, 512], dt.bfloat16, tag="st", name=f"st_{i}")
            nc.vector.tensor_copy(st[:], tp[:])
            ag_ap = agin[:, :]
            dst = bass.AP(ag_ap.tensor, i * 512 * P, [[512, P], [1, 512]])
            nc.sync.dma_start(dst, st[:])

        nc.gpsimd.collective_compute(
            "AllGather", mybir.AluOpType.bypass,
            replica_groups=[list(range(NCORE))],
            ins=[agin[:, :].opt()], outs=[table[:, :].opt()])

        # ---------------- phase 2: edge stream -----------------------------
        x2cur = [None]

        def finish_window(seg, w):
            sl = staging[:, w * WIN:(w + 1) * WIN]
            if seg == 0:
                nc.vector.tensor_copy(sl, x2cur[0][:])
            else:
                nc.vector.tensor_add(sl, sl, x2cur[0][:])
            x2cur[0] = None

        for ci, (seg, b0, b1) in enumerate(chunks):
            nb = b1 - b0
            Gt = edge.tile([P, GCH * P], dt.bfloat16, tag="G", name=f"G{ci}")
            gt_ap = Gt[:, :]
            g_out = bass.AP(gt_ap.tensor, gt_ap.offset,
                            [[gt_ap.ap[0][0], P], [P, nb], [1, P]])
            src = table[0:TBL_SPLIT, :] if seg == 0 else table[TBL_SPLIT:NAPG, :]
            nc.gpsimd.dma_gather(
                g_out, src, gidx_sb[:, b0 * 8:b1 * 8],
                num_idxs=nb * P, num_idxs_reg=nb * P, elem_size=P,
                single_packet=False)
            B8 = edge.tile([P, GCH * P], dt.int8, tag="B8", name=f"B8{ci}")
            nc.sync.dma_start(B8[:, :nb * P], bfti[:, b0 * P:b1 * P])
            Bt = edge.tile([P, GCH * P], dt.bfloat16, tag="B", name=f"B{ci}")
            nc.vector.tensor_copy(Bt[:, :nb * P], B8[:, :nb * P])

            for q0 in range(0, nb, 4):
                qn = min(4, nb - q0)
                mm = psM.tile([P, 512], dt.float32, tag="mm",
                              name=f"mm{ci}_{q0}")
                for j in range(qn):
                    nc.tensor.matmul(
                        mm[:, j * P:(j + 1) * P],
                        Bt[:, (q0 + j) * P:(q0 + j + 1) * P],
                        Wbf, start=True, stop=True)
                xg = xoh.tile([P, 512], dt.bfloat16, tag="x",
                              name=f"x{ci}_{q0}")
                nc.vector.tensor_mul(xg[:, :qn * P],
                                     Gt[:, q0 * P:(q0 + qn) * P],
                                     mm[:, :qn * P])
                # 4 onehot blocks in one DVE op via stride-0 broadcast APs:
                # oh4[p, j*W+e] = (iota[e] == tcol[p, b0+q0+j])
                oh4 = xoh.tile([P, 512], dt.bfloat16, tag="oh",
                               name=f"oh{ci}_{q0}")
                in0 = bass.AP(iota_sb.tensor, iota_sb.offset,
                              [[iota_sb.ap[0][0], P], [0, qn], [1, WIN]])
                tsl = tcol_sb[:, b0 + q0:b0 + q0 + qn]
                in1 = bass.AP(tsl.tensor, tsl.offset,
                              [[tsl.ap[0][0], P], [1, qn], [0, WIN]])
                nc.vector.tensor_tensor(oh4[:, :qn * WIN], in0, in1,
                                        mybir.AluOpType.is_equal)
                for j in range(qn):
                    b = b0 + q0 + j
                    _, w, first, last = blocks[b]
                    if first:
                        x2cur[0] = psX.tile([P, WIN], dt.float32, tag="x2",
                                            name=f"x2_{b}")
                    nc.tensor.matmul(x2cur[0][:],
                                     xg[:, j * P:(j + 1) * P],
                                     oh4[:, j * WIN:(j + 1) * WIN],
                                     start=first, stop=last)
                    if last:
                        finish_window(seg, w)

        # ---------------- phase 3: atom MLP (transposed) --------------------
        wptr, gi = 0, 0
        while wptr < NWIN:
            nw = min(4, NWIN - wptr)
            ncols = nw * WIN
            col0 = wptr * WIN
            rhs = staging[:, col0:col0 + ncols]
            p3 = psA.tile([P, 512], dt.float32, tag="p1", name=f"p3_{gi}")
            nc.tensor.matmul(p3[:, :ncols], Wmp, rhs, start=True, stop=True)
            xv = mlp.tile([P, 512], dt.bfloat16, tag="mx", name=f"mx_{gi}")
            nc.scalar.activation(xv[:, :ncols], p3[:, :ncols],
                                 ACT)
            for i in range(3):
                Ai, Bi = W[4 + 2 * i], W[5 + 2 * i]
                pa = psA.tile([P, 512], dt.float32, tag="p1",
                              name=f"pa{gi}_{i}")
                nc.tensor.matmul(pa[:, :ncols], Ai, xv[:, :ncols],
                                 start=True, stop=True)
                ad = mlp.tile([P, 512], dt.bfloat16, tag="ad",
                              name=f"ad{gi}_{i}")
                nc.scalar.activation(ad[:, :ncols], pa[:, :ncols],
                                     ACT)
                pb = psA.tile([P, 512], dt.float32, tag="p1",
                              name=f"pb{gi}_{i}")
                nc.tensor.matmul(pb[:, :ncols], Bi, ad[:, :ncols],
                                 start=True, stop=True)
                bd = mlp.tile([P, 512], dt.bfloat16, tag="bd",
                              name=f"bd{gi}_{i}")
                nc.scalar.activation(bd[:, :ncols], pb[:, :ncols],
                                     ACT)
                tsum = mlp.tile([P, 512], dt.bfloat16, tag="ts",
                                name=f"ts{gi}_{i}")
                nc.vector.tensor_add(tsum[:, :ncols], xv[:, :ncols],
                                     bd[:, :ncols])
                if i < 2:
                    xv = mlp.tile([P, 512], dt.bfloat16, tag="mx",
                                  name=f"mx{gi}_{i}")
                    nc.vector.tensor_scalar(xv[:, :ncols], tsum[:, :ncols],
                                            INV_SQRT2, None,
                                            mybir.AluOpType.mult)
                else:
                    ov = mlp.tile([P, 512], dt.bfloat16, tag="ov",
                                  name=f"ov{gi}")
                    nc.vector.tensor_scalar(ov[:, :ncols], tsum[:, :ncols],
                                            INV_SQRT2 * SILU_S, None,
                                            mybir.AluOpType.mult)
                    nc.sync.dma_start(outt[:, col0:col0 + ncols],
                                      ov[:, :ncols])
            wptr += nw
            gi += 1

    nc.compile()
    return nc


def prepare(h, bf, idx_s, idx_t, w_bf, w_pre, w_mlp1, w_res, scale_sum,
            enable_asserts=False):
    """Pack inputs + build the compiled SPMD program. Returns (nc, in_maps)."""
    pk = pack_edges(idx_s, idx_t)
    in_maps = build_host_inputs(np.asarray(h), np.asarray(bf),
                                np.asarray(w_bf), np.asarray(w_pre),
                                np.asarray(w_mlp1), np.asarray(w_res),
                                np.asarray(scale_sum), pk)
    nc = build_bass(pk, enable_asserts=enable_asserts)
    return nc, in_maps


def unshard_output(per_core_outt):
    out = np.empty((NA, EMB), np.float32)
    for c in range(NCORE):
        t = np.asarray(per_core_outt[c]).astype(np.float32)
        out[c * APC:(c + 1) * APC] = t[:, :APC].T
    return out


def kernel(h, bf, idx_s, idx_t, w_bf, w_pre, w_mlp1, w_res, scale_sum):
    nc, in_maps = prepare(h, bf, idx_s, idx_t, w_bf, w_pre, w_mlp1, w_res,
                          scale_sum)
    res = run_bass_kernel_spmd(nc, in_maps, list(range(NCORE)))
    return unshard_output([res.results[c]["outt"] for c in range(NCORE)])


# revision 17
# speedup vs baseline: 1.1102x; 1.1102x over previous
"""Trainium2 Bass kernel for nn_HadamardBlock (GNN message passing block).

Reference computation (see reference.py):
    h_res = residual_layer(h, w_pre0, w_pre1)            # (nAtoms, E)
    mlp_bf = bf @ w_bf                                   # (nEdges, E)
    x = h_res[idx_s] * mlp_bf                            # gather + Hadamard
    x2 = segment_sum(x, idx_t, nAtoms) * scale_sum
    out = MLP(x2)   # Dense+ScaledSiLU then 3 residual blocks

Distribution strategy (8 cores, SPMD):
  - Edges are sharded by OWNER OF TARGET ATOM (atom ranges of 6250/core),
    so segment_sum is fully core-local and the atom MLP is data-parallel.
  - Phase 1 (h_res table) is sharded: each core computes 13 of the 104
    padded 512-atom tiles and an HBM-HBM AllGather replicates the full
    (53248, 128) bf16 table to every core.
  - Edge features ship as int8 (bf quantized by *127; 1/127 folded into
    w_bf) and are converted int8->bf16 on the vector engine on device.
  - The source gather h_res[idx_s] uses DMA gather (int16 indices; the
    table is addressed in two halves split at row 32768, and each core's
    edge stream is grouped low-half-first so indices fit in int16).
  - segment_sum runs on the tensor engine as x2^T += x^T @ onehot over
    128-atom windows; onehot is built by one DVE tensor_scalar(is_equal)
    per 128-edge block against an iota constant.
  - Per-(window, half) edge slot capacities are data-driven (max count
    over cores, rounded to 128) to minimize padding bytes; the program
    is rebuilt per call, which the fast walrus BIR->NEFF compiler makes
    cheap (~0.3 s).

Everything is sized to minimize bytes shipped through the axon tunnel:
host->device upload is the dominant cost of a run in this environment
(~70 MB/s), not device execution (~1 ms).
"""

import math
import os
import sys
from contextlib import ExitStack

import numpy as np

for _p in ("/opt/trn_rl_repo", "/root/.axon_site/_ro/trn_rl_repo"):
    if os.path.isdir(_p) and _p not in sys.path:
        sys.path.insert(0, _p)

import ml_dtypes

import concourse.bacc as bacc
import concourse.bass as bass
import concourse.mybir as mybir
import concourse.tile as tile
from concourse.bass_utils import run_bass_kernel_spmd

BF16 = ml_dtypes.bfloat16
F32 = np.float32

P = 128
NA = 50000          # atoms
NE = 800000         # edges
EMB = 128
NCORE = 8
APC = NA // NCORE   # atoms per core = 6250
WIN = 128           # scatter window (atoms) = onehot width
NWIN = (APC + WIN - 1) // WIN           # 49 windows/core
TILE = 512
TPC = 13            # phase-1 tiles per core (104 total >= 98 real)
NAPC = TPC * TILE   # 6656 atom slots computed per core
NAPG = NCORE * NAPC  # 53248 global padded table rows
TBL_SPLIT = 32768   # table row split so int16 gather indices stay in range
GCH = 64            # gather/bfT chunk size in 128-edge blocks
QBF = 127.0         # bf int8 quantization scale
QH = 40.0           # h int8 quantization scale (h^T/S clipped at +-3.175)
SILU_S = 1.0 / 0.6
INV_SQRT2 = float(1.0 / math.sqrt(2.0))

dt = mybir.dt


def _ceil128(x):
    return (np.asarray(x, np.int64) + 127) // 128 * 128


def _atom_perm(a):
    """Atom id -> physical row in the h_res DRAM table.

    Phase 1 stores each 512-atom tile via 4 PE transposes packed contiguously
    per partition; row q = tile*512 + (r%128)*4 + r//128 for r = a%512."""
    a = np.asarray(a, np.int64)
    i, r = a // 512, a % 512
    return i * 512 + (r % 128) * 4 + r // 128


def pack_edges(idx_s, idx_t):
    """Host-side edge sharding/padding. Returns static structure (identical
    across cores) + per-core slot assignment of every real edge."""
    idx_s = np.asarray(idx_s, np.int64)
    idx_t = np.asarray(idx_t, np.int64)
    core = idx_t // APC
    tloc = idx_t - core * APC
    w = tloc // WIN
    trel = tloc - w * WIN
    pi = _atom_perm(idx_s)
    g = (pi >= TBL_SPLIT).astype(np.int64)

    key = (core * 2 + g) * NWIN + w
    order = np.argsort(key, kind="stable")
    cnt = np.bincount(key, minlength=NCORE * 2 * NWIN).reshape(NCORE, 2, NWIN)

    # data-driven per-window capacities (walrus compiles per call anyway,
    # so an input-dependent program costs nothing and saves padding bytes)
    LCAP = np.maximum(_ceil128(cnt[:, 0, :].max(axis=0)), 128)
    HCAP = np.maximum(_ceil128(cnt[:, 1, :].max(axis=0)), 128)

    low_off = np.concatenate([[0], np.cumsum(LCAP)])
    HBASE = int(low_off[-1])
    high_off = HBASE + np.concatenate([[0], np.cumsum(HCAP)])
    EPAD = int(high_off[-1])

    off_by_key = np.empty(NCORE * 2 * NWIN, np.int64)
    for c in range(NCORE):
        off_by_key[(c * 2 + 0) * NWIN:(c * 2 + 1) * NWIN] = low_off[:-1]
        off_by_key[(c * 2 + 1) * NWIN:(c * 2 + 2) * NWIN] = high_off[:-1]
    grp_start = np.concatenate([[0], np.cumsum(cnt.reshape(-1))])
    k_sorted = key[order]
    pos = np.arange(NE, dtype=np.int64) - grp_start[k_sorted]
    # slot in ORIGINAL edge order (avoids materializing permuted copies of
    # the big edge-feature array later)
    slot = np.empty(NE, np.int64)
    slot[order] = off_by_key[k_sorted] + pos

    return dict(
        core=core, slot=slot, pi=pi, g=g, trel=trel,
        LCAP=LCAP.astype(int), HCAP=HCAP.astype(int),
        EPAD=EPAD, HBASE=HBASE, NBLK=EPAD // 128,
    )


def build_host_inputs(h, bf, w_bf, w_pre, w_mlp1, w_res, scale_sum, pk):
    """Build the per-core in_maps (numpy arrays keyed by DRAM tensor name)."""
    S = SILU_S
    EPAD, NBLK = pk["EPAD"], pk["NBLK"]

    # folded weights, natural [in, out] layout; 10 slots of [128,128]:
    #  0: W0' = S*w_pre0       1: W1' = S*w_pre1
    #  2: Wm' = S*C*scale*w_mlp1        3: w_bf/QBF (bf int8 dequant folded)
    #  4..9: Ai' = S*w_res[i,0], Bi' = S*w_res[i,1]
    scale = float(np.asarray(scale_sum).reshape(-1)[0])
    wl = [
        np.asarray(w_pre[0], F32) * S,
        np.asarray(w_pre[1], F32) * S,
        np.asarray(w_mlp1, F32) * (S * INV_SQRT2 * scale),
        np.asarray(w_bf, F32) * (1.0 / QBF),
    ]
    for i in range(3):
        wl.append(np.asarray(w_res[i, 0], F32) * S)
        wl.append(np.asarray(w_res[i, 1], F32) * S)
    wts = np.concatenate([x.astype(BF16) for x in wl], axis=1)  # [128, 10*128]

    # h^T/S quantized to int8 at fixed scale QH (clips |h| beyond ~5.3 sigma)
    hq = np.zeros((P, NAPG), np.int8)
    hq[:, :NA] = np.clip(
        np.rint(np.asarray(h, F32).T * (QH / S)), -127, 127).astype(np.int8)

    iota = np.ascontiguousarray(
        np.broadcast_to(np.arange(WIN, dtype=F32).astype(BF16), (P, WIN)))
    ident = np.eye(P, dtype=BF16)

    # bf -> int8 in chunks (values in [0,1); round(bf*127) fits exactly);
    # chunking keeps the f32 temporary small on the cold path
    bf = np.asarray(bf, F32)
    bf_q = np.empty((NE, P), np.int8)
    tmp = np.empty((100000, P), F32)
    for s in range(0, NE, 100000):
        e = min(s + 100000, NE)
        t = tmp[:e - s]
        np.multiply(bf[s:e], QBF, out=t)
        t += 0.5
        bf_q[s:e] = t.astype(np.int8)

    ecore, slot = pk["core"], pk["slot"]
    bfr = np.zeros((NCORE, EPAD, P), np.int8)
    bfr[ecore, slot] = bf_q

    gidx = np.zeros((NCORE, EPAD), np.int16)
    gidx[ecore, slot] = (pk["pi"] - pk["g"] * TBL_SPLIT).astype(np.int16)
    gidx = np.ascontiguousarray(
        gidx.reshape(NCORE, EPAD // 16, 16).transpose(0, 2, 1))  # [NCORE,16,EPAD//16]

    tcol = np.zeros((NCORE, EPAD), BF16)
    tcol[ecore, slot] = pk["trel"].astype(BF16)
    tcol = tcol.reshape(NCORE, NBLK, P)

    # single bf16 aux tensor: wts | iota | ident | tcol  -> one device_put
    in_maps = []
    for c in range(NCORE):
        aux = np.empty((P, 10 * P + WIN + P + NBLK), BF16)
        aux[:, :10 * P] = wts
        aux[:, 10 * P:10 * P + WIN] = iota
        aux[:, 10 * P + WIN:10 * P + WIN + P] = ident
        np.copyto(aux[:, 10 * P + WIN + P:], tcol[c].T)
        bfti = np.empty((P, EPAD + NAPC), np.int8)
        np.copyto(bfti[:, :EPAD], bfr[c].T)
        bfti[:, EPAD:] = hq[:, c * NAPC:(c + 1) * NAPC]
        in_maps.append({
            "aux": aux, "bfti": bfti,
            "gidx": np.ascontiguousarray(gidx[c]),
        })
    return in_maps


def blocks_static(pk):
    """Static per-block schedule: list of (seg, w, start, stop)."""
    blocks = []
    for seg, CAPS in ((0, pk["LCAP"]), (1, pk["HCAP"])):
        for w in range(NWIN):
            nb = CAPS[w] // 128
            for j in range(nb):
                blocks.append((seg, w, j == 0, j == nb - 1))
    return blocks


def chunks_static(pk):
    """Gather/bfT chunk list: (seg, b0, b1) block ranges within one table
    half, at most GCH blocks each."""
    blocks = blocks_static(pk)
    chunks = []
    b = 0
    while b < len(blocks):
        seg = blocks[b][0]
        e = b
        while e < len(blocks) and blocks[e][0] == seg and e - b < GCH:
            e += 1
        chunks.append((seg, b, e))
        b = e
    return chunks


def build_bass(pk, enable_asserts=False, act_fn=None):
    EPAD, NBLK = pk["EPAD"], pk["NBLK"]
    blocks = blocks_static(pk)
    chunks = chunks_static(pk)
    ACT = act_fn or mybir.ActivationFunctionType.Silu

    nc = bacc.Bacc("TRN2", target_bir_lowering=False, debug=False,
                   enable_asserts=enable_asserts, num_devices=NCORE)

    AUXW = 10 * P + WIN + P + NBLK
    aux = nc.dram_tensor("aux", [P, AUXW], dt.bfloat16, kind="ExternalInput").ap()
    bfti = nc.dram_tensor("bfti", [P, EPAD + NAPC], dt.int8,
                          kind="ExternalInput").ap()
    gidx = nc.dram_tensor("gidx", [16, EPAD // 16], dt.int16,
                          kind="ExternalInput").ap()
    outt = nc.dram_tensor("outt", [P, NWIN * WIN], dt.bfloat16,
                          kind="ExternalOutput").ap()

    with tile.TileContext(nc) as tc, ExitStack() as ctx:
        const = ctx.enter_context(tc.tile_pool(name="const", bufs=1))
        dram = ctx.enter_context(tc.tile_pool(name="dram", bufs=1, space="DRAM"))
        ph1 = ctx.enter_context(tc.tile_pool(name="ph1", bufs=3))
        edge = ctx.enter_context(tc.tile_pool(name="edge", bufs=2))
        xoh = ctx.enter_context(tc.tile_pool(name="xoh", bufs=4))
        mlp = ctx.enter_context(tc.tile_pool(name="mlp", bufs=2))
        psA = ctx.enter_context(tc.tile_pool(name="psA", bufs=2, space="PSUM"))
        psT = ctx.enter_context(tc.tile_pool(name="psT", bufs=2, space="PSUM"))
        psM = ctx.enter_context(tc.tile_pool(name="psM", bufs=2, space="PSUM"))
        psX = ctx.enter_context(tc.tile_pool(name="psX", bufs=2, space="PSUM"))

        # resident constants / streams (one DMA for the whole aux block)
        aux_sb = const.tile([P, AUXW], dt.bfloat16)
        nc.sync.dma_start(aux_sb[:], aux[:])
        W = [aux_sb[:, i * P:(i + 1) * P] for i in range(10)]
        W0p, W1p, Wmp, Wbf = W[0], W[1], W[2], W[3]
        iota_sb = aux_sb[:, 10 * P:10 * P + WIN]
        ident_sb = aux_sb[:, 10 * P + WIN:10 * P + WIN + P]
        tcol16 = aux_sb[:, 10 * P + WIN + P:AUXW]
        tcol_sb = const.tile([P, NBLK], dt.float32)
        nc.vector.tensor_copy(tcol_sb[:], tcol16)
        # gather indices arrive 16-wrapped; replicate to the 128-partition
        # layout the SWDGE gather engine expects
        gidx_sb = const.tile([P, EPAD // 16], dt.int16)
        for k in range(8):
            nc.sync.dma_start(gidx_sb[16 * k:16 * (k + 1), :], gidx[:])
        staging = const.tile([P, NWIN * WIN], dt.bfloat16)

        agin = dram.tile([NAPC, P], dt.bfloat16, tag="agin")
        table = dram.tile([NAPG, P], dt.bfloat16, tag="table")

        # -------- phase 1: h_res table (sharded + AllGather) ---------------
        for i in range(TPC):
            h8 = ph1.tile([P, 512], dt.int8, tag="h8", name=f"h8_{i}")
            nc.sync.dma_start(
                h8[:], bfti[:, EPAD + i * 512:EPAD + (i + 1) * 512])
            hT = ph1.tile([P, 512], dt.bfloat16, tag="hT", name=f"hT{i}")
            nc.vector.tensor_scalar(hT[:], h8[:], 1.0 / QH, None,
                                    mybir.AluOpType.mult)
            p1 = psA.tile([P, 512], dt.float32, tag="p1", name=f"p1_{i}")
            nc.tensor.matmul(p1[:], W0p, hT[:], start=True, stop=True)
            y1 = ph1.tile([P, 512], dt.bfloat16, tag="y1", name=f"y1_{i}")
            nc.scalar.activation(y1[:], p1[:], ACT)
            p2 = psA.tile([P, 512], dt.float32, tag="p1", name=f"p2_{i}")
            nc.tensor.matmul(p2[:], W1p, y1[:], start=True, stop=True)
            y2 = ph1.tile([P, 512], dt.bfloat16, tag="y2", name=f"y2_{i}")
            nc.scalar.activation(y2[:], p2[:], ACT)
            tres = ph1.tile([P, 512], dt.bfloat16, tag="tres", name=f"tr_{i}")
            nc.vector.tensor_add(tres[:], hT[:], y2[:])
            tp = psT.tile([P, 512], dt.bfloat16, tag="tp", name=f"tp_{i}")
            for t in range(4):
                nc.tensor.transpose(tp[:, t * P:(t + 1) * P],
                                    tres[:, t * P:(t + 1) * P], ident_sb)
            st = ph1.tile([P, 512], dt.bfloat16, tag="st", name=f"st_{i}")
            nc.vector.tensor_copy(st[:], tp[:])
            ag_ap = agin[:, :]
            dst = bass.AP(ag_ap.tensor, i * 512 * P, [[512, P], [1, 512]])
            nc.sync.dma_start(dst, st[:])

        nc.gpsimd.collective_compute(
            "AllGather", mybir.AluOpType.bypass,
            replica_groups=[list(range(NCORE))],
            ins=[agin[:, :].opt()], outs=[table[:, :].opt()])

        # ---------------- phase 2: edge stream -----------------------------
        x2cur = [None]

        def finish_window(seg, w):
            sl = staging[:, w * WIN:(w + 1) * WIN]
            if seg == 0:
                nc.vector.tensor_copy(sl, x2cur[0][:])
            else:
                nc.vector.tensor_add(sl, sl, x2cur[0][:])
            x2cur[0] = None

        for ci, (seg, b0, b1) in enumerate(chunks):
            nb = b1 - b0
            Gt = edge.tile([P, GCH * P], dt.bfloat16, tag="G", name=f"G{ci}")
            gt_ap = Gt[:, :]
            g_out = bass.AP(gt_ap.tensor, gt_ap.offset,
                            [[gt_ap.ap[0][0], P], [P, nb], [1, P]])
            src = table[0:TBL_SPLIT, :] if seg == 0 else table[TBL_SPLIT:NAPG, :]
            nc.gpsimd.dma_gather(
                g_out, src, gidx_sb[:, b0 * 8:b1 * 8],
                num_idxs=nb * P, num_idxs_reg=nb * P, elem_size=P,
                single_packet=False)
            B8 = edge.tile([P, GCH * P], dt.int8, tag="B8", name=f"B8{ci}")
            nc.sync.dma_start(B8[:, :nb * P], bfti[:, b0 * P:b1 * P])
            Bt = edge.tile([P, GCH * P], dt.bfloat16, tag="B", name=f"B{ci}")
            nc.vector.tensor_copy(Bt[:, :nb * P], B8[:, :nb * P])

            for q0 in range(0, nb, 4):
                qn = min(4, nb - q0)
                mm = psM.tile([P, 512], dt.float32, tag="mm",
                              name=f"mm{ci}_{q0}")
                for j in range(qn):
                    nc.tensor.matmul(
                        mm[:, j * P:(j + 1) * P],
                        Bt[:, (q0 + j) * P:(q0 + j + 1) * P],
                        Wbf, start=True, stop=True)
                xg = xoh.tile([P, 512], dt.bfloat16, tag="x",
                              name=f"x{ci}_{q0}")
                nc.vector.tensor_mul(xg[:, :qn * P],
                                     Gt[:, q0 * P:(q0 + qn) * P],
                                     mm[:, :qn * P])
                # 4 onehot blocks in one DVE op via stride-0 broadcast APs:
                # oh4[p, j*W+e] = (iota[e] == tcol[p, b0+q0+j])
                oh4 = xoh.tile([P, 512], dt.bfloat16, tag="oh",
                               name=f"oh{ci}_{q0}")
                in0 = bass.AP(iota_sb.tensor, iota_sb.offset,
                              [[iota_sb.ap[0][0], P], [0, qn], [1, WIN]])
                tsl = tcol_sb[:, b0 + q0:b0 + q0 + qn]
                in1 = bass.AP(tsl.tensor, tsl.offset,
                              [[tsl.ap[0][0], P], [1, qn], [0, WIN]])
                nc.vector.tensor_tensor(oh4[:, :qn * WIN], in0, in1,
                                        mybir.AluOpType.is_equal)
                for j in range(qn):
                    b = b0 + q0 + j
                    _, w, first, last = blocks[b]
                    if first:
                        x2cur[0] = psX.tile([P, WIN], dt.float32, tag="x2",
                                            name=f"x2_{b}")
                    nc.tensor.matmul(x2cur[0][:],
                                     xg[:, j * P:(j + 1) * P],
                                     oh4[:, j * WIN:(j + 1) * WIN],
                                     start=first, stop=last)
                    if last:
                        finish_window(seg, w)

        # ---------------- phase 3: atom MLP (transposed) --------------------
        wptr, gi = 0, 0
        while wptr < NWIN:
            nw = min(4, NWIN - wptr)
            ncols = nw * WIN
            col0 = wptr * WIN
            rhs = staging[:, col0:col0 + ncols]
            p3 = psA.tile([P, 512], dt.float32, tag="p1", name=f"p3_{gi}")
            nc.tensor.matmul(p3[:, :ncols], Wmp, rhs, start=True, stop=True)
            xv = mlp.tile([P, 512], dt.bfloat16, tag="mx", name=f"mx_{gi}")
            nc.scalar.activation(xv[:, :ncols], p3[:, :ncols],
                                 ACT)
            for i in range(3):
                Ai, Bi = W[4 + 2 * i], W[5 + 2 * i]
                pa = psA.tile([P, 512], dt.float32, tag="p1",
                              name=f"pa{gi}_{i}")
                nc.tensor.matmul(pa[:, :ncols], Ai, xv[:, :ncols],
                                 start=True, stop=True)
                ad = mlp.tile([P, 512], dt.bfloat16, tag="ad",
                              name=f"ad{gi}_{i}")
                nc.scalar.activation(ad[:, :ncols], pa[:, :ncols],
                                     ACT)
                pb = psA.tile([P, 512], dt.float32, tag="p1",
                              name=f"pb{gi}_{i}")
                nc.tensor.matmul(pb[:, :ncols], Bi, ad[:, :ncols],
                                 start=True, stop=True)
                bd = mlp.tile([P, 512], dt.bfloat16, tag="bd",
                              name=f"bd{gi}_{i}")
                nc.scalar.activation(bd[:, :ncols], pb[:, :ncols],
                                     ACT)
                tsum = mlp.tile([P, 512], dt.bfloat16, tag="ts",
                                name=f"ts{gi}_{i}")
                nc.vector.tensor_add(tsum[:, :ncols], xv[:, :ncols],
                                     bd[:, :ncols])
                if i < 2:
                    xv = mlp.tile([P, 512], dt.bfloat16, tag="mx",
                                  name=f"mx{gi}_{i}")
                    nc.vector.tensor_scalar(xv[:, :ncols], tsum[:, :ncols],
                                            INV_SQRT2, None,
                                            mybir.AluOpType.mult)
                else:
                    ov = mlp.tile([P, 512], dt.bfloat16, tag="ov",
                                  name=f"ov{gi}")
                    nc.vector.tensor_scalar(ov[:, :ncols], tsum[:, :ncols],
                                            INV_SQRT2 * SILU_S, None,
                                            mybir.AluOpType.mult)
                    nc.sync.dma_start(outt[:, col0:col0 + ncols],
                                      ov[:, :ncols])
            wptr += nw
            gi += 1

    nc.compile()
    return nc


def prepare(h, bf, idx_s, idx_t, w_bf, w_pre, w_mlp1, w_res, scale_sum,
            enable_asserts=False):
    """Pack inputs + build the compiled SPMD program. Returns (nc, in_maps)."""
    pk = pack_edges(idx_s, idx_t)
    in_maps = build_host_inputs(np.asarray(h), np.asarray(bf),
                                np.asarray(w_bf), np.asarray(w_pre),
                                np.asarray(w_mlp1), np.asarray(w_res),
                                np.asarray(scale_sum), pk)
    nc = build_bass(pk, enable_asserts=enable_asserts)
    return nc, in_maps


def unshard_output(per_core_outt):
    out = np.empty((NA, EMB), np.float32)
    for c in range(NCORE):
        t = np.asarray(per_core_outt[c]).astype(np.float32)
        out[c * APC:(c + 1) * APC] = t[:, :APC].T
    return out


def kernel(h, bf, idx_s, idx_t, w_bf, w_pre, w_mlp1, w_res, scale_sum):
    nc, in_maps = prepare(h, bf, idx_s, idx_t, w_bf, w_pre, w_mlp1, w_res,
                          scale_sum)
    res = run_bass_kernel_spmd(nc, in_maps, list(range(NCORE)))
    return unshard_output([res.results[c]["outt"] for c in range(NCORE)])
